# revision 53
# baseline (speedup 1.0000x reference)
import time

import numpy as np

import concourse.bacc as bacc
import concourse.mybir as mybir
import concourse.tile as tile
from concourse.bass_utils import run_bass_kernel_spmd

F32 = mybir.dt.float32
F32R = mybir.dt.float32r
BF16 = mybir.dt.bfloat16
AF = mybir.ActivationFunctionType
OP = mybir.AluOpType

FULL = dict(B=2, T=4096, D=2048, H=32, KV=8, DH=64, W=1024, BASE=10000.0)
BIGNEG = -1e30


def _derived(cfg):
    d = dict(cfg)
    d["CH"] = cfg["T"] // 4
    d["KB"] = cfg["W"] // 128
    d["DT"] = cfg["D"] // 128
    d["NP"] = cfg["H"] // 2
    d["NC"] = [(i, min(512, d["CH"] - i)) for i in range(0, d["CH"], 512)]
    assert d["NP"] * 128 == cfg["D"] and d["NP"] % 4 == 0
    return d


def build(cfg):
    c = _derived(cfg)
    CH, KB, DT, NP, KV, H = c["CH"], c["KB"], c["DT"], c["NP"], c["KV"], c["H"]
    NC = c["NC"]
    hpkv = H // KV
    OH = c["D"] // 2
    AUG = 64 + KB
    nc = bacc.Bacc("TRN2", target_bir_lowering=False, debug=False)

    xT = nc.dram_tensor("xT", [128, DT, CH], BF16, kind="ExternalInput")
    wqT = nc.dram_tensor("wqT", [128, NP, DT, 128], BF16, kind="ExternalInput")
    woT = nc.dram_tensor("woT", [128, NP, c["D"]], BF16, kind="ExternalInput")
    kaug = nc.dram_tensor("kaug", [AUG, KV, KB, 128], BF16,
                          kind="ExternalInput")
    vaug = nc.dram_tensor("vaug", [128, KV, KB, 65], BF16,
                          kind="ExternalInput")
    cosT = nc.dram_tensor("cosT", [128, CH], BF16, kind="ExternalInput")
    sinT = nc.dram_tensor("sinT", [128, CH], BF16, kind="ExternalInput")
    tri = nc.dram_tensor("tri", [128, 128], BF16, kind="ExternalInput")
    brow = nc.dram_tensor("brow", [KB, CH], BF16, kind="ExternalInput")
    sel2 = nc.dram_tensor("sel2", [128, 256], BF16, kind="ExternalInput")
    out = nc.dram_tensor("out", [CH, c["D"]], F32, kind="ExternalOutput")

    swap = [i ^ 1 for i in range(32)]

    with nc.allow_low_precision(reason="bf16 matmuls are intended"), \
         tile.TileContext(nc) as tc:
        with (
            tc.tile_pool(name="consts", bufs=1) as cp,
            tc.tile_pool(name="qa", bufs=8) as qap,
            tc.tile_pool(name="at", bufs=1) as atp,
            tc.tile_pool(name="wo", bufs=1) as wop,
            tc.tile_pool(name="psq", bufs=2, space="PSUM") as psq,
        ):
            qas = {}
            wqs = {}
            ats = []
            den_pending = []

            def phase_a_alloc(m, br_sb):
                wq_m = wqs.pop(m)
                qaA = qap.tile([AUG, CH], BF16, tag="qa")
                qaB = qap.tile([AUG, CH], BF16, tag="qa")
                qas[m] = (qaA, qaB)
                nc.sync.dma_start(qaA[64:AUG, :], br_sb[:])
                nc.sync.dma_start(qaB[64:AUG, :], br_sb[:])
                return wq_m

            def phase_a_chunk(m, wq_m, n0, nn, rp, xts, cos_sb, sin_sb, psq):
                qaA, qaB = qas[m]
                qp = psq.tile([128, nn], F32, tag="qp", name=f"qp{m}_{n0}")
                for kt in range(DT):
                    nc.tensor.matmul(
                        qp[:], wq_m[:, kt, :], xts[:, kt, n0:n0 + nn],
                        start=(kt == 0), stop=(kt == DT - 1))
                qcp = rp.tile([128, nn], BF16, tag="qcp")
                nc.scalar.copy(qcp[:], qp[:])
                t1 = rp.tile([128, nn], BF16, tag="t1")
                nc.vector.tensor_mul(t1[:], qp[:], cos_sb[:, n0:n0 + nn])
                qs = rp.tile([128, nn], BF16, tag="qs")
                nc.vector.stream_shuffle(qs[:], qcp[:], swap)
                t2 = rp.tile([128, nn], BF16, tag="t2")
                nc.vector.tensor_mul(t2[:], qs[:], sin_sb[:, n0:n0 + nn])
                nc.vector.tensor_add(qaA[0:64, n0:n0 + nn],
                                     t1[0:64, :], t2[0:64, :])
                nc.vector.tensor_add(qaB[0:64, n0:n0 + nn],
                                     t1[64:128, :], t2[64:128, :])

            def phase_a(m, rp, xts, cos_sb, sin_sb, br_sb, psq):
                wq_m = phase_a_alloc(m, br_sb)
                for n0, nn in NC:
                    phase_a_chunk(m, wq_m, n0, nn, rp, xts, cos_sb, sin_sb,
                                  psq)

            def phase_b(m, ep, rcp, consts, pss, psa, filler=None):
                ka_sb, va_sb, tri_sb, sel2_sb = consts
                qaA, qaB = qas.pop(m)
                kv0 = (2 * m) // hpkv
                kv1 = (2 * m + 1) // hpkv
                at = atp.tile([128, CH], BF16, tag=f"at{m}")
                ats.append(at)
                for n0, nn in NC:
                    avA = psa.tile([128, nn], F32, tag="avA",
                                   name=f"avA{m}_{n0}")
                    avB = psa.tile([128, nn], F32, tag="avB",
                                   name=f"avB{m}_{n0}")
                    pend = []
                    FL = min(2, KB - 1)
                    for kb in range(KB):
                        if kb == FL:
                            if filler is not None:
                                filler(n0, nn)
                            while den_pending:
                                den_pending.pop(0)()
                        sp = pss.tile([128, 2 * nn], F32, tag="sp",
                                      name=f"sp{m}_{n0}_{kb}")
                        nc.tensor.matmul(sp[:, 0:nn], ka_sb[:, kv0, kb, :],
                                         qaA[:, n0:n0 + nn],
                                         start=True, stop=True)
                        nc.tensor.matmul(sp[:, nn:2 * nn],
                                         ka_sb[:, kv1, kb, :],
                                         qaB[:, n0:n0 + nn],
                                         start=True, stop=True)
                        er = ep.tile([128, 2 * nn], BF16, tag="er")
                        nc.scalar.activation(er[:], sp[:], AF.Exp)
                        if n0 <= 128 * kb < n0 + nn:
                            d0 = 128 * kb - n0
                            nc.gpsimd.tensor_mul(
                                er[:, d0:d0 + 128], er[:, d0:d0 + 128],
                                tri_sb[:])
                            nc.gpsimd.tensor_mul(
                                er[:, nn + d0:nn + d0 + 128],
                                er[:, nn + d0:nn + d0 + 128], tri_sb[:])

                        def av(kb=kb, er=er):
                            nc.tensor.matmul(
                                avA[0:65, :], va_sb[:, kv0, kb, :],
                                er[:, 0:nn],
                                start=(kb == 0), stop=(kb == KB - 1))
                            nc.tensor.matmul(
                                avB[0:65, :], va_sb[:, kv1, kb, :],
                                er[:, nn:2 * nn],
                                start=(kb == 0), stop=(kb == KB - 1))
                        if len(pend) >= 2:
                            pend.pop(0)()
                        pend.append(av)
                    for f in pend:
                        f()

                    RA = rcp.tile([128, nn], BF16, tag="recA")
                    nc.vector.reciprocal(RA[0:1, :], avA[64:65, :])
                    RB = rcp.tile([128, nn], BF16, tag="recB")
                    nc.vector.reciprocal(RB[0:1, :], avB[64:65, :])
                    RD = rcp.tile([128, nn], BF16, tag="recD")
                    nc.vector.tensor_sub(RD[0:1, :], RB[0:1, :], RA[0:1, :])

                    def den(avA=avA, avB=avB, n0=n0, nn=nn, at=at,
                            RA=RA, RD=RD):
                        bct = pss.tile([128, 2 * nn], F32, tag="sp",
                                       name=f"bc{m}_{n0}")
                        nc.tensor.matmul(bct[:, 0:nn], sel2_sb[0:1, 0:128],
                                         RA[0:1, :], start=True, stop=False)
                        nc.tensor.matmul(bct[:, 0:nn],
                                         sel2_sb[0:1, 128:256],
                                         RD[0:1, :], start=False, stop=True)
                        bcs = rcp.tile([128, nn], BF16, tag="bcs")
                        nc.scalar.copy(bcs[:], bct[:, 0:nn])
                        nc.vector.tensor_mul(at[0:64, n0:n0 + nn],
                                             avA[0:64, :], bcs[0:64, :])
                        nc.vector.tensor_mul(at[64:128, n0:n0 + nn],
                                             avB[0:64, :], bcs[64:128, :])
                    den_pending.append(den)

            with (
                tc.tile_pool(name="ab", bufs=1) as abp,
                tc.tile_pool(name="wq", bufs=3) as wp,
                tc.tile_pool(name="rope", bufs=4) as rp,
                tc.tile_pool(name="expp", bufs=8) as ep,
                tc.tile_pool(name="rec", bufs=2) as rcp,
                tc.tile_pool(name="pss", bufs=2, space="PSUM") as pss,
                tc.tile_pool(name="psav", bufs=1, space="PSUM") as psa,
            ):
                def load_wq(m):
                    t = wp.tile([128, DT, 128], BF16, tag="wq")
                    nc.sync.dma_start(t[:], wqT[:, m, :, :])
                    wqs[m] = t

                load_wq(0)
                load_wq(1)
                xts = abp.tile([128, DT, CH], BF16)
                for kt in range(DT):
                    nc.sync.dma_start(xts[:, kt, :], xT[:, kt, :])
                cos_sb = abp.tile([128, CH], BF16)
                nc.sync.dma_start(cos_sb[:], cosT[:])
                sin_sb = abp.tile([128, CH], BF16)
                nc.sync.dma_start(sin_sb[:], sinT[:])
                br_sb = cp.tile([KB, CH], BF16)
                nc.sync.dma_start(br_sb[:], brow[:])
                tri_sb = cp.tile([128, 128], BF16)
                nc.sync.dma_start(tri_sb[:], tri[:])
                ka_sb = cp.tile([AUG, KV, KB, 128], BF16)
                nc.sync.dma_start(ka_sb[:, 0:1], kaug[:, 0:1])
                va_sb = cp.tile([128, KV, KB, 65], BF16)
                nc.sync.dma_start(va_sb[:, 0:1], vaug[:, 0:1])
                phase_a(0, rp, xts, cos_sb, sin_sb, br_sb, psq)
                load_wq(2)
                if KV > 1:
                    nc.sync.dma_start(ka_sb[:, 1:KV], kaug[:, 1:KV])
                    nc.sync.dma_start(va_sb[:, 1:KV], vaug[:, 1:KV])
                sel2_sb = cp.tile([128, 256], BF16)
                nc.sync.dma_start(sel2_sb[:], sel2[:])
                wo_h0 = wop.tile([128, NP, OH], BF16, tag="wo0")
                nc.sync.dma_start(wo_h0[:], woT[:, :, 0:OH])
                consts = (ka_sb, va_sb, tri_sb, sel2_sb)
                opx_pre = []
                for m in range(NP):
                    if m + 2 < NP:
                        load_wq(m + 2)
                    if m + 1 < NP:
                        wq_n = phase_a_alloc(m + 1, br_sb)

                        def filler(n0, nn, m2=m + 1, wq_n=wq_n):
                            phase_a_chunk(m2, wq_n, n0, nn, rp, xts,
                                          cos_sb, sin_sb, psq)
                    else:
                        def filler(n0, nn):
                            o0 = 512 * len(opx_pre)
                            ow = min(512, OH - o0)
                            if o0 >= OH:
                                return
                            opx = psq.tile([128, ow], F32, tag="qp",
                                           name=f"opre{o0}")
                            for kq in range(NP - 2):
                                nc.tensor.matmul(
                                    opx[:], ats[kq][:, 0:128],
                                    wo_h0[:, kq, o0:o0 + ow],
                                    start=(kq == 0), stop=False)
                            opx_pre.append((opx, o0, ow))
                    phase_b(m, ep, rcp, consts, pss, psa, filler)
                while den_pending:
                    den_pending.pop(0)()

            with (
                tc.tile_pool(name="osb", bufs=3) as op_,
                tc.tile_pool(name="wo2", bufs=1) as wop2,
                tc.tile_pool(name="psc", bufs=4, space="PSUM") as psc,
            ):
                wo_h1 = wop2.tile([128, NP, OH], BF16, tag="wo1")
                nc.sync.dma_start(wo_h1[:], woT[:, :, OH:2 * OH])
                MQ = CH // 128
                pre = {(0, 0, o0): (opx, ow) for (opx, o0, ow) in opx_pre}
                last = (1, MQ - 1, ((OH - 1) // 512) * 512)
                for nh in range(2):
                    wo_h = wo_h0 if nh == 0 else wo_h1
                    for mq in range(MQ):
                        qsl = slice(128 * mq, 128 * (mq + 1))
                        for o0 in range(0, OH, 512):
                            ow = min(512, OH - o0)
                            if (nh, mq, o0) in pre:
                                opx, ow = pre[(nh, mq, o0)]
                                kq0 = NP - 2
                            else:
                                opx = psc.tile([128, ow], F32, tag="opx")
                                kq0 = 0
                            for kq in range(kq0, NP):
                                nc.tensor.matmul(
                                    opx[:], ats[kq][:, qsl],
                                    wo_h[:, kq, o0:o0 + ow],
                                    start=(kq == 0), stop=(kq == NP - 1))
                            hw = ow // 2 if (nh, mq, o0) == last else ow
                            for p0 in range(0, ow, hw):
                                pw = min(hw, ow - p0)
                                osb = op_.tile([128, pw], F32, tag="os")
                                nc.vector.tensor_copy(osb[:],
                                                      opx[:, p0:p0 + pw])
                                c0 = OH * nh + o0 + p0
                                nc.sync.dma_start(
                                    out[qsl, c0:c0 + pw], osb[:])
    nc.compile()
    return nc


def host_inputs(cfg, x, k_cache, v_cache, Wq, Wo, core):
    import ml_dtypes
    bf16 = ml_dtypes.bfloat16
    c = _derived(cfg)
    CH, KB, KV, W, DH, DT, NP = (c["CH"], c["KB"], c["KV"], c["W"], c["DH"],
                                 c["DT"], c["NP"])
    b, ch = core // 4, core % 4
    Tc = k_cache.shape[2]
    f32 = np.float32

    xchunk = x[b, CH * ch:CH * (ch + 1), :].T.astype(f32)
    xT = np.ascontiguousarray(
        xchunk.reshape(DT, 128, CH).transpose(1, 0, 2)).astype(bf16)
    wq_s = (Wq.T.astype(f32) * f32(1.0 / np.sqrt(DH)))
    wqT = np.ascontiguousarray(
        wq_s.reshape(DT, 128, NP, 128).transpose(1, 2, 0, 3)).astype(bf16)
    woT = np.ascontiguousarray(
        Wo.T.astype(f32).reshape(NP, 128, c["D"]).transpose(1, 0, 2)
    ).astype(bf16)
    kw = k_cache[b, :, Tc - W:, :].astype(f32)
    kT64 = kw.reshape(KV, KB, 128, DH).transpose(3, 0, 1, 2)
    kaugm = np.zeros((64 + KB, KV, KB, 128), f32)
    kaugm[0:64] = kT64
    for kb in range(KB):
        kaugm[64 + kb, :, kb, :] = 1.0
    vw = v_cache[b, :, Tc - W:, :].astype(f32).reshape(KV, KB, 128, DH)
    vp = vw.transpose(2, 0, 1, 3)
    vaugm = np.ones((128, KV, KB, 65), f32)
    vaugm[:, :, :, :DH] = vp
    pos = (CH * ch + np.arange(CH)).astype(f32)
    inv = 1.0 / (cfg["BASE"] ** (np.arange(0, DH, 2, dtype=f32) / DH))
    r = np.arange(128)
    u = (r % 64) // 2
    ang = pos[None, :] * inv[u][:, None]
    cosT = np.cos(ang).astype(bf16)
    sinT = (np.sin(ang) * np.where(r % 2 == 0, -1.0, 1.0)[:, None]
            ).astype(bf16)
    if ch == 0:
        trim = (np.arange(128)[:, None] <= np.arange(128)[None, :]
                ).astype(f32)
        browm = np.zeros((KB, CH), f32)
        for kb in range(KB):
            browm[kb, :128 * kb] = BIGNEG
    else:
        trim = np.ones((128, 128), f32)
        browm = np.zeros((KB, CH), f32)
    sel2 = np.zeros((128, 256), f32)
    sel2[0, 0:128] = 1.0
    sel2[0, 192:256] = 1.0
    return {"xT": xT, "wqT": wqT, "woT": woT,
            "kaug": kaugm.astype(bf16), "vaug": vaugm.astype(bf16),
            "cosT": cosT, "sinT": sinT,
            "tri": trim.astype(bf16), "brow": browm.astype(bf16),
            "sel2": sel2.astype(bf16)}


_NC_CACHE = {}


def run(cfg, x, k_cache, v_cache, Wq, Wo, trace=False):
    key = tuple(sorted((k, v) for k, v in cfg.items()))
    if key not in _NC_CACHE:
        _NC_CACHE[key] = build(cfg)
    nc = _NC_CACHE[key]
    in_maps = [host_inputs(cfg, x, k_cache, v_cache, Wq, Wo, c)
               for c in range(8)]
    res = None
    for attempt in range(3):
        try:
            res = run_bass_kernel_spmd(nc, in_maps, core_ids=list(range(8)),
                                       trace=trace)
            break
        except Exception:
            if attempt == 2:
                raise
            time.sleep(2.0)
    outs = [res.results[c]["out"] for c in range(8)]
    full = np.stack([np.concatenate(outs[0:4], axis=0),
                     np.concatenate(outs[4:8], axis=0)])
    return full, res


def kernel(x, k_cache, v_cache, Wq, Wo):
    full, _ = run(FULL, np.asarray(x), np.asarray(k_cache),
                  np.asarray(v_cache), np.asarray(Wq), np.asarray(Wo))
    return full.astype(np.float32)
